# revision 16
# baseline (speedup 1.0000x reference)
"""Block attention (no softmax) Trainium2 Bass kernel.

Problem: x:[8,8192,128] -> q,k,v projections -> per-256-block attention with
a +/-255-row K/V window, NO softmax, -> out:[8,8192,128].

Key algebraic identity: with no softmax, (Q K^T * s) V == (Q * s) (K^T V).
Per window n, M_n = sum_{r in win(n)} k_r v_r^T is a [128,128] matrix; then
out_blk = (Q_blk * s) @ M_n.  This turns the [256x766] score matrices into
[128x128] K^T V accumulations, ~6x fewer FLOPs and no big score tensors.

Sharding: batch (8) across the 8 NeuronCores, data-parallel, no halo
exchange (windows never cross batch boundaries).

Engine budget (PE is pinned at 1.2 GHz on this part — no HAM warmup):
  PE:   64 fp16 transposes + 16 qT (N=512) + 64 kv (N=256) + ~188 window
        K^T V chunks + 64 out matmuls
  DVE:  xT copies, k|v bias adds, out copies (batched [128,512] PSUM reads)
  ACT:  fp32->fp16 casts of x, qT bias+scale, M casts
  GPSIMD: zeroed-row0 k-chunk copies (PE base-partition workaround)
"""

import sys
from contextlib import ExitStack

import numpy as np

for _p in ("/opt/trn_rl_repo", "/root/.axon_site/_ro/trn_rl_repo"):
    if _p not in sys.path:
        sys.path.append(_p)

import concourse.bass as bass
import concourse.tile as tile
from concourse import bacc, mybir
from concourse.bass_utils import run_bass_kernel_spmd

S = 8192          # sequence length per batch/core
D = 128           # input dim
H = 128           # hidden dim
BS = 256          # block size
HALO = 255        # window_size - 1
NB = S // BS      # 32 blocks
NCORES = 8
SCALE = float(1.0 / np.sqrt(np.float32(D)))

F32 = mybir.dt.float32
F16 = mybir.dt.float16
F32R = mybir.dt.float32r
CDT = F16  # matmul operand dtype (PSUM accumulation is always fp32)
AF = mybir.ActivationFunctionType


def _window_chunks(n):
    """128-aligned contraction chunks covering window n's valid rows.

    Window n covers rows [256n-255, 256n+511) clipped to [0, S).  All chunk
    starts are ==0 or ==1 (mod 128), so each chunk lives inside one
    128-partition group: returns (chunk_idx, p0, p1) triples.
    """
    lo = max(0, BS * n - HALO)
    hi = min(S, BS * n + BS + HALO)
    chunks = []
    a = lo
    while a < hi:
        b = min(hi, (a // 128 + 1) * 128)
        chunks.append((a // 128, a % 128, a % 128 + (b - a)))
        a = b
    return chunks


def build_nc():
    nc = bacc.Bacc(
        "TRN2",
        target_bir_lowering=False,
        debug=False,
        enable_asserts=False,
        num_devices=NCORES,
    )

    x = nc.dram_tensor("x", [S, D], F32, kind="ExternalInput").ap()
    cf32 = nc.dram_tensor("cf32", [128, 641], F32, kind="ExternalInput").ap()
    cf16 = nc.dram_tensor("cf16", [128, 3 * H], CDT, kind="ExternalInput").ap()
    out = nc.dram_tensor("out", [S, H], F32, kind="ExternalOutput").ap()

    xv = x.rearrange("(c p) d -> p c d", p=128)       # [128, 64, 128]
    out_t = out.rearrange("(c p) h -> p c h", p=128)  # [128, 64, 128]

    with ExitStack() as ctx:
        tc = ctx.enter_context(tile.TileContext(nc))
        const = ctx.enter_context(tc.tile_pool(name="const", bufs=1))
        cf32_sb = const.tile([128, 641], F32)
        nc.sync.dma_start(cf32_sb, cf32)
        cf16_sb = const.tile([128, 3 * H], CDT)
        nc.sync.dma_start(cf16_sb, cf16)
        id_sb = cf32_sb[:, 0:128]
        bq_sb = cf32_sb[:, 128:129]
        bkv_sb = cf32_sb[:, 129:641].rearrange("p (a b) -> p a b", a=2)
        wq_sb = cf16_sb[:, 0:H]
        wkv_sb = cf16_sb[:, H : 3 * H]

        big = ctx.enter_context(tc.tile_pool(name="big", bufs=1))
        qT_all = big.tile([128, S], CDT)            # q^T, scaled, [h, s]
        kv_all = big.tile([128, S // 128, 2 * H], CDT)  # [p, chunk, k|v]
        # Copies of even k-chunks with row 0 zeroed: window head-chunks start
        # at partition 1, which the PE can't address (base partition must be
        # 0/32/64) — a zeroed row 0 contributes nothing to K^T V instead.
        kz_all = big.tile([128, 31, H], CDT)

        xn_pool = ctx.enter_context(tc.tile_pool(name="xn", bufs=6))
        xT_pool = ctx.enter_context(tc.tile_pool(name="xT", bufs=3))
        m_pool = ctx.enter_context(tc.tile_pool(name="m", bufs=3))
        o_pool = ctx.enter_context(tc.tile_pool(name="o", bufs=3))
        psum = ctx.enter_context(
            tc.tile_pool(name="ps", bufs=8, space=bass.MemorySpace.PSUM)
        )

        # ---- Phase A+B: load x, cast, transpose, project q/k/v -------------
        for ci in range(S // 512):
            xn4 = xn_pool.tile([128, 4, 128], F32, tag="xn")
            nc.sync.dma_start(xn4, xv[:, 4 * ci : 4 * ci + 4, :])
            psA = psum.tile([128, 512], F32, tag="ps", name="psA")
            xT = xT_pool.tile([128, 512], CDT, tag="xT")
            for j in range(4):
                nc.tensor.transpose(
                    psA[:, 128 * j : 128 * (j + 1)], xn4[:, j, :], id_sb
                )
                # per-chunk cast so kv/qT matmuls unblock incrementally
                nc.vector.tensor_copy(
                    xT[:, 128 * j : 128 * (j + 1)],
                    psA[:, 128 * j : 128 * (j + 1)],
                )

            # q^T chunk: [h, 512] = wq_t.T @ xT ; bias+scale fused on ACT copy
            psQ = psum.tile([128, 512], F32, tag="ps", name="psQ")
            nc.tensor.matmul(psQ, wq_sb, xT, start=True, stop=True)
            nc.scalar.activation(
                qT_all[:, 512 * ci : 512 * (ci + 1)],
                psQ,
                AF.Identity,
                bias=bq_sb,
                scale=SCALE,
            )

            # k|v chunks: [s128, 256] = xT_j.T @ [wk_t | wv_t]; bias on DVE
            for h in range(2):  # pairs of 128-chunks -> one [128,512] PSUM
                psKV = psum.tile([128, 2, 2 * H], F32, tag="ps", name="psKV")
                for j2 in range(2):
                    j = 2 * h + j2
                    nc.tensor.matmul(
                        psKV[:, j2, :],
                        xT[:, 128 * j : 128 * (j + 1)],
                        wkv_sb,
                        start=True,
                        stop=True,
                    )
                cc = 4 * ci + 2 * h
                nc.vector.tensor_add(kv_all[:, cc : cc + 2, :], psKV, bkv_sb)
                if cc <= 60:
                    nc.gpsimd.tensor_copy(
                        kz_all[:, cc // 2, :], kv_all[:, cc, 0:H]
                    )
                    nc.gpsimd.memset(kz_all[0:1, cc // 2, :], 0.0)

        # ---- Phase C+D interleaved: per window pair t = rows [512t,512t+512)
        # M_{2t}, M_{2t+1} = K^T V; then out rows via qT.T @ M.  Interleaving
        # spreads the out DMAs across the phase instead of a tail.
        for t in range(NB // 2):
            psM = psum.tile([128, 2, 128], F32, tag="ps", name="psM")
            for w in range(2):
                n = 2 * t + w
                chunks = _window_chunks(n)
                for i, (c, p0, p1) in enumerate(chunks):
                    if p0 == 1:
                        # head chunk: zeroed-row0 copy, full 128 rows
                        lhs = kz_all[:, c // 2, :]
                        rhs = kv_all[:, c, H : 2 * H]
                    else:
                        lhs = kv_all[p0:p1, c, 0:H]
                        rhs = kv_all[p0:p1, c, H : 2 * H]
                    nc.tensor.matmul(
                        psM[:, w, :], lhs, rhs,
                        start=(i == 0),
                        stop=(i == len(chunks) - 1),
                    )
            m2 = m_pool.tile([128, 2, 128], CDT, tag="m")
            nc.scalar.copy(m2, psM)

            psO = psum.tile([128, 4, 128], F32, tag="ps", name="psO")
            for w in range(4):
                n, j = divmod(4 * t + w, 2)
                s0 = BS * n + 128 * j
                nc.tensor.matmul(
                    psO[:, w, :],
                    qT_all[:, s0 : s0 + 128],
                    m2[:, n - 2 * t, :],
                    start=True,
                    stop=True,
                )
            ostage = o_pool.tile([128, 4, 128], F32, tag="o")
            nc.vector.tensor_copy(ostage, psO)
            nc.gpsimd.dma_start(out_t[:, 4 * t : 4 * t + 4, :], ostage)

    nc.compile()
    return nc


_NC_CACHE = None


def _get_nc():
    global _NC_CACHE
    if _NC_CACHE is None:
        _NC_CACHE = build_nc()
    return _NC_CACHE


def _make_in_maps(inputs):
    x = np.ascontiguousarray(np.asarray(inputs["x"], dtype=np.float32))
    Wq = np.asarray(inputs["Wq"], dtype=np.float32)
    Wk = np.asarray(inputs["Wk"], dtype=np.float32)
    Wv = np.asarray(inputs["Wv"], dtype=np.float32)
    bq = np.asarray(inputs["bq"], dtype=np.float32)
    bk = np.asarray(inputs["bk"], dtype=np.float32)
    bv = np.asarray(inputs["bv"], dtype=np.float32)

    wdt = np.float16 if CDT == F16 else np.float32
    cf16 = np.concatenate([Wq.T, Wk.T, Wv.T], axis=1).astype(wdt)
    # ACT computes func(in*scale + bias), so the q bias ships pre-scaled
    bq_col = (bq * SCALE).reshape(H, 1).astype(np.float32)
    bkv_row = np.concatenate([bk, bv])
    bkv_rep = np.broadcast_to(
        np.tile(bkv_row, 2)[None, :], (128, 4 * H)
    ).astype(np.float32)
    ident = np.eye(128, dtype=np.float32)
    cf32 = np.concatenate([ident, bq_col, bkv_rep], axis=1)

    shared = {
        "cf32": np.ascontiguousarray(cf32),
        "cf16": np.ascontiguousarray(cf16),
    }
    return [{"x": np.ascontiguousarray(x[c]), **shared} for c in range(NCORES)]


def kernel(**inputs):
    nc = _get_nc()
    in_maps = _make_in_maps(inputs)
    res = run_bass_kernel_spmd(nc, in_maps, core_ids=list(range(NCORES)))
    return np.stack([res.results[c]["out"] for c in range(NCORES)], axis=0)


def run_traced(inputs):
    """Like kernel() but with NTFF tracing; returns (out, BassKernelResults)."""
    nc = _get_nc()
    in_maps = _make_in_maps(inputs)
    res = run_bass_kernel_spmd(
        nc, in_maps, core_ids=list(range(NCORES)), trace=True
    )
    out = np.stack([res.results[c]["out"] for c in range(NCORES)], axis=0)
    return out, res


# revision 17
# speedup vs baseline: 1.3317x; 1.3317x over previous
"""Block attention (no softmax) Trainium2 Bass kernel.

Problem: x:[8,8192,128] -> q,k,v projections -> per-256-block attention with
a +/-255-row K/V window, NO softmax, -> out:[8,8192,128].

Key algebraic identity: with no softmax, (Q K^T * s) V == (Q * s) (K^T V).
Per window n, M_n = sum_{r in win(n)} k_r v_r^T is a [128,128] matrix; then
out_blk = (Q_blk * s) @ M_n.  This turns the [256x766] score matrices into
[128x128] K^T V accumulations, ~6x fewer FLOPs and no big score tensors.

Sharding: batch (8) across the 8 NeuronCores, data-parallel, no halo
exchange (windows never cross batch boundaries).

Engine budget (PE is pinned at 1.2 GHz on this part — no HAM warmup):
  PE:   64 fp16 transposes + 16 qT (N=512) + 64 kv (N=256) + ~188 window
        K^T V chunks + 64 out matmuls
  DVE:  xT copies, k|v bias adds, out copies (batched [128,512] PSUM reads)
  ACT:  fp32->fp16 casts of x, qT bias+scale, M casts
  GPSIMD: zeroed-row0 k-chunk copies (PE base-partition workaround)
"""

import sys
from contextlib import ExitStack

import numpy as np

for _p in ("/opt/trn_rl_repo", "/root/.axon_site/_ro/trn_rl_repo"):
    if _p not in sys.path:
        sys.path.append(_p)

import concourse.bass as bass
import concourse.tile as tile
from concourse import bacc, mybir
from concourse.bass_utils import run_bass_kernel_spmd

S = 8192          # sequence length per batch/core
D = 128           # input dim
H = 128           # hidden dim
BS = 256          # block size
HALO = 255        # window_size - 1
NB = S // BS      # 32 blocks
NCORES = 8
SCALE = float(1.0 / np.sqrt(np.float32(D)))

F32 = mybir.dt.float32
F16 = mybir.dt.float16
F32R = mybir.dt.float32r
CDT = F16  # matmul operand dtype (PSUM accumulation is always fp32)
AF = mybir.ActivationFunctionType


def _window_chunks(n):
    """128-aligned contraction chunks covering window n's valid rows.

    Window n covers rows [256n-255, 256n+511) clipped to [0, S).  All chunk
    starts are ==0 or ==1 (mod 128), so each chunk lives inside one
    128-partition group: returns (chunk_idx, p0, p1) triples.
    """
    lo = max(0, BS * n - HALO)
    hi = min(S, BS * n + BS + HALO)
    chunks = []
    a = lo
    while a < hi:
        b = min(hi, (a // 128 + 1) * 128)
        chunks.append((a // 128, a % 128, a % 128 + (b - a)))
        a = b
    return chunks


def build_nc():
    nc = bacc.Bacc(
        "TRN2",
        target_bir_lowering=False,
        debug=False,
        enable_asserts=False,
        num_devices=NCORES,
    )

    x = nc.dram_tensor("x", [S, D], F32, kind="ExternalInput").ap()
    cf32 = nc.dram_tensor("cf32", [128, 641], F32, kind="ExternalInput").ap()
    cf16 = nc.dram_tensor("cf16", [128, 3 * H], CDT, kind="ExternalInput").ap()
    out = nc.dram_tensor("out", [S, H], F32, kind="ExternalOutput").ap()

    xv = x.rearrange("(c p) d -> p c d", p=128)       # [128, 64, 128]
    out_t = out.rearrange("(c p) h -> p c h", p=128)  # [128, 64, 128]

    with ExitStack() as ctx:
        tc = ctx.enter_context(tile.TileContext(nc))
        const = ctx.enter_context(tc.tile_pool(name="const", bufs=1))
        cf32_sb = const.tile([128, 641], F32)
        nc.sync.dma_start(cf32_sb, cf32)
        cf16_sb = const.tile([128, 3 * H], CDT)
        nc.sync.dma_start(cf16_sb, cf16)
        id_sb = cf32_sb[:, 0:128]
        bq_sb = cf32_sb[:, 128:129]
        bkv_sb = cf32_sb[:, 129:641].rearrange("p (a b) -> p a b", a=2)
        wq_sb = cf16_sb[:, 0:H]
        wkv_sb = cf16_sb[:, H : 3 * H]

        big = ctx.enter_context(tc.tile_pool(name="big", bufs=1))
        qT_all = big.tile([128, S], CDT)            # q^T, scaled, [h, s]
        kv_all = big.tile([128, S // 128, 2 * H], CDT)  # [p, chunk, k|v]
        # Copies of even k-chunks with row 0 zeroed: window head-chunks start
        # at partition 1, which the PE can't address (base partition must be
        # 0/32/64) — a zeroed row 0 contributes nothing to K^T V instead.
        kz_all = big.tile([128, 31, H], CDT)

        xn_pool = ctx.enter_context(tc.tile_pool(name="xn", bufs=6))
        xT_pool = ctx.enter_context(tc.tile_pool(name="xT", bufs=3))
        m_pool = ctx.enter_context(tc.tile_pool(name="m", bufs=3))
        o_pool = ctx.enter_context(tc.tile_pool(name="o", bufs=3))
        psum = ctx.enter_context(
            tc.tile_pool(name="ps", bufs=8, space=bass.MemorySpace.PSUM)
        )

        # ---- Phase A+B: load x, cast, transpose, project q/k/v -------------
        for ci in range(S // 512):
            xn4 = xn_pool.tile([128, 4, 128], F32, tag="xn")
            nc.sync.dma_start(xn4, xv[:, 4 * ci : 4 * ci + 4, :])
            xT = xT_pool.tile([128, 512], CDT, tag="xT")
            for j in range(4):
                # separate PSUM tile per chunk: a shared bank would make
                # Tile serialize PE writes against the DVE cast reads
                psA = psum.tile([128, 128], F32, tag="ps", name="psA")
                nc.tensor.transpose(psA, xn4[:, j, :], id_sb)
                # per-chunk cast so kv/qT matmuls unblock incrementally
                nc.vector.tensor_copy(xT[:, 128 * j : 128 * (j + 1)], psA)

            # q^T chunk: [h, 512] = wq_t.T @ xT ; bias+scale fused on ACT copy
            psQ = psum.tile([128, 512], F32, tag="ps", name="psQ")
            nc.tensor.matmul(psQ, wq_sb, xT, start=True, stop=True)
            nc.scalar.activation(
                qT_all[:, 512 * ci : 512 * (ci + 1)],
                psQ,
                AF.Identity,
                bias=bq_sb,
                scale=SCALE,
            )

            # k|v chunks: [s128, 256] = xT_j.T @ [wk_t | wv_t]; bias on DVE
            for h in range(2):  # pairs of 128-chunks -> one [128,512] PSUM
                psKV = psum.tile([128, 2, 2 * H], F32, tag="ps", name="psKV")
                for j2 in range(2):
                    j = 2 * h + j2
                    nc.tensor.matmul(
                        psKV[:, j2, :],
                        xT[:, 128 * j : 128 * (j + 1)],
                        wkv_sb,
                        start=True,
                        stop=True,
                    )
                cc = 4 * ci + 2 * h
                nc.vector.tensor_add(kv_all[:, cc : cc + 2, :], psKV, bkv_sb)
                if cc <= 60:
                    nc.gpsimd.tensor_copy(
                        kz_all[:, cc // 2, :], kv_all[:, cc, 0:H]
                    )
                    nc.gpsimd.memset(kz_all[0:1, cc // 2, :], 0.0)

        # ---- Phase C+D interleaved: per window pair t = rows [512t,512t+512)
        # M_{2t}, M_{2t+1} = K^T V; then out rows via qT.T @ M.  Interleaving
        # spreads the out DMAs across the phase instead of a tail.
        for t in range(NB // 2):
            psM = psum.tile([128, 2, 128], F32, tag="ps", name="psM")
            for w in range(2):
                n = 2 * t + w
                chunks = _window_chunks(n)
                for i, (c, p0, p1) in enumerate(chunks):
                    if p0 == 1:
                        # head chunk: zeroed-row0 copy, full 128 rows
                        lhs = kz_all[:, c // 2, :]
                        rhs = kv_all[:, c, H : 2 * H]
                    else:
                        lhs = kv_all[p0:p1, c, 0:H]
                        rhs = kv_all[p0:p1, c, H : 2 * H]
                    nc.tensor.matmul(
                        psM[:, w, :], lhs, rhs,
                        start=(i == 0),
                        stop=(i == len(chunks) - 1),
                    )
            m2 = m_pool.tile([128, 2, 128], CDT, tag="m")
            nc.scalar.copy(m2, psM)

            psO = psum.tile([128, 4, 128], F32, tag="ps", name="psO")
            for w in range(4):
                n, j = divmod(4 * t + w, 2)
                s0 = BS * n + 128 * j
                nc.tensor.matmul(
                    psO[:, w, :],
                    qT_all[:, s0 : s0 + 128],
                    m2[:, n - 2 * t, :],
                    start=True,
                    stop=True,
                )
            ostage = o_pool.tile([128, 4, 128], F32, tag="o")
            nc.vector.tensor_copy(ostage, psO)
            nc.gpsimd.dma_start(out_t[:, 4 * t : 4 * t + 4, :], ostage)

    nc.compile()
    return nc


_NC_CACHE = None


def _get_nc():
    global _NC_CACHE
    if _NC_CACHE is None:
        _NC_CACHE = build_nc()
    return _NC_CACHE


def _make_in_maps(inputs):
    x = np.ascontiguousarray(np.asarray(inputs["x"], dtype=np.float32))
    Wq = np.asarray(inputs["Wq"], dtype=np.float32)
    Wk = np.asarray(inputs["Wk"], dtype=np.float32)
    Wv = np.asarray(inputs["Wv"], dtype=np.float32)
    bq = np.asarray(inputs["bq"], dtype=np.float32)
    bk = np.asarray(inputs["bk"], dtype=np.float32)
    bv = np.asarray(inputs["bv"], dtype=np.float32)

    wdt = np.float16 if CDT == F16 else np.float32
    cf16 = np.concatenate([Wq.T, Wk.T, Wv.T], axis=1).astype(wdt)
    # ACT computes func(in*scale + bias), so the q bias ships pre-scaled
    bq_col = (bq * SCALE).reshape(H, 1).astype(np.float32)
    bkv_row = np.concatenate([bk, bv])
    bkv_rep = np.broadcast_to(
        np.tile(bkv_row, 2)[None, :], (128, 4 * H)
    ).astype(np.float32)
    ident = np.eye(128, dtype=np.float32)
    cf32 = np.concatenate([ident, bq_col, bkv_rep], axis=1)

    shared = {
        "cf32": np.ascontiguousarray(cf32),
        "cf16": np.ascontiguousarray(cf16),
    }
    return [{"x": np.ascontiguousarray(x[c]), **shared} for c in range(NCORES)]


def kernel(**inputs):
    nc = _get_nc()
    in_maps = _make_in_maps(inputs)
    res = run_bass_kernel_spmd(nc, in_maps, core_ids=list(range(NCORES)))
    return np.stack([res.results[c]["out"] for c in range(NCORES)], axis=0)


def run_traced(inputs):
    """Like kernel() but with NTFF tracing; returns (out, BassKernelResults)."""
    nc = _get_nc()
    in_maps = _make_in_maps(inputs)
    res = run_bass_kernel_spmd(
        nc, in_maps, core_ids=list(range(NCORES)), trace=True
    )
    out = np.stack([res.results[c]["out"] for c in range(NCORES)], axis=0)
    return out, res
